# revision 2
# baseline (speedup 1.0000x reference)
"""Transformer encoder layer (post-norm, 16 heads, d_model=1024, d_ff=4096)
on 8 Trainium2 NeuronCores — v2.

Sharding: batch(4) x seq-half(2) -> 8 shards (fully local, no collectives).
Each core: K/V for its batch's FULL sequence, Q/attention/FFN/LN for its
1024-query half.

v2 over v1:
- fp8e4 (max 240) + DoubleRow matmuls for QKV projections, attention apply
  (E x V) and O-projection: 2x PE throughput. Scores and FFN stay bf16
  (numpy sim: fp8 FFN pushes rel err past the 2e-2 gate; fp8 attention adds
  only ~2e-3 because attention output is small vs the residual stream).
- exp computed as exp(score/8 - 1.5) so E fits fp8 (softmax ratio invariant).
- V-bias folded into the O-projection bias host-side (convex softmax weights)
  so V's evacuation is a single rescale.
- O-proj psum carries c1 = S_V*swo scale; the residual xh is pre-scaled
  host-side and LayerNorm is scale-invariant (eps scaled to match).
- QKV interleaved with attention per 128-feature chunk (KT[d] then heads of
  chunk d-1) so ACT exp overlaps PE matmuls.
- Evacuations on DVE (tensor_scalar mult+add), exp on ACT; LN2 tail all-DVE.
- DMA order: x query-half + Wq first so the first matmul starts ~6us in.
"""

import numpy as np
import ml_dtypes

B, S, D = 4, 2048, 1024
H, DK = 16, 64
DFF = 4096
SQ = S // 2          # queries per core
P = 128              # partitions
EPS = 1e-6
NCORES = 8

KC = D // P          # 8 contraction chunks of 128
DCH = D // P         # 8 feature chunks
SCH = S // P         # 16 key chunks
SQCH = SQ // P       # 8 query chunks
NW = 512

S_V = 16.0           # fp8 storage scale for V (and thus concatT = S_V*attn)
E_BIAS = -1.5        # exp(score/8 + E_BIAS): keeps E under fp8e4 max 240

BF16 = ml_dtypes.bfloat16
F8 = ml_dtypes.float8_e4m3

_PROG = None


def _build_program():
    import concourse.bacc as bacc
    import concourse.tile as tile
    import concourse.mybir as mybir
    from concourse.masks import make_identity
    import concourse.bass as bass

    f32 = mybir.dt.float32
    bf16 = mybir.dt.bfloat16
    fp8 = mybir.dt.float8e4
    AF = mybir.ActivationFunctionType
    Alu = mybir.AluOpType
    DR = mybir.MatmulPerfMode.DoubleRow

    nc = bacc.Bacc("TRN2", target_bir_lowering=False, debug=False,
                   num_devices=NCORES)

    # ---- DRAM parameters (per-core shards supplied by host) ----
    xt = nc.declare_dram_parameter("xt", [D, S], fp8, isOutput=False)       # sx*x[b].T
    wq = nc.declare_dram_parameter("wq", [D, D], fp8, isOutput=False)       # swq[col]*Wq
    xh = nc.declare_dram_parameter("xh", [SQ, D], f32, isOutput=False)      # c1*(x+bo+bv@Wo)
    wk = nc.declare_dram_parameter("wk", [D, D], fp8, isOutput=False)
    wv = nc.declare_dram_parameter("wv", [D, D], fp8, isOutput=False)
    wo = nc.declare_dram_parameter("wo", [D, D], fp8, isOutput=False)
    w1 = nc.declare_dram_parameter("w1", [D, DFF], bf16, isOutput=False)    # alpha1*W1
    w2 = nc.declare_dram_parameter("w2", [DFF, D], bf16, isOutput=False)
    rq = nc.declare_dram_parameter("rq", [D], f32, isOutput=False)          # 1/(sx*swq)
    rk = nc.declare_dram_parameter("rk", [D], f32, isOutput=False)
    rv = nc.declare_dram_parameter("rv", [P], f32, isOutput=False)          # S_V/(sx*swv)
    eps1 = nc.declare_dram_parameter("eps1", [P], f32, isOutput=False)      # c1*EPS
    bq = nc.declare_dram_parameter("bq", [D], f32, isOutput=False)
    bk = nc.declare_dram_parameter("bk", [D], f32, isOutput=False)
    b1p = nc.declare_dram_parameter("b1", [DFF], f32, isOutput=False)       # b1+bias1@W1
    a1p = nc.declare_dram_parameter("alpha1", [D], f32, isOutput=False)
    g1p = nc.declare_dram_parameter("beta1", [D], f32, isOutput=False)      # bias1+b2
    a2p = nc.declare_dram_parameter("alpha2", [D], f32, isOutput=False)
    g2p = nc.declare_dram_parameter("beta2", [D], f32, isOutput=False)
    out = nc.declare_dram_parameter("out", [SQ, D], f32, isOutput=True)

    def bcast(ap_1d, n):
        return bass.AP(tensor=ap_1d.tensor, offset=ap_1d.offset,
                       ap=[[0, P]] + list(ap_1d.ap[-1:]))[:, 0:n]

    with tile.TileContext(nc) as tc:
        with tc.tile_pool(name="main", bufs=1) as mp, \
             tc.tile_pool(name="stream", bufs=2) as sp, \
             tc.tile_pool(name="ktp", bufs=2) as ktp, \
             tc.tile_pool(name="small", bufs=4) as smp, \
             tc.tile_pool(name="at2p", bufs=14) as at2p, \
             tc.tile_pool(name="tokp", bufs=3) as tokp, \
             tc.tile_pool(name="ps", bufs=4, space="PSUM") as ps, \
             tc.tile_pool(name="psat", bufs=2, space="PSUM") as psat, \
             tc.tile_pool(name="pstr", bufs=2, space="PSUM") as pstr:

            # ---- first wave of DMAs: unblock QT as fast as possible ----
            xtb = mp.tile([P, KC, S], fp8, tag="slotC")
            nc.sync.dma_start(
                out=xtb[:, :, 0:SQ],
                in_=xt[:, 0:SQ].rearrange("(c p) s -> p c s", p=P))
            wq_sb = sp.tile([P, KC, D], fp8, tag="slotE")
            nc.sync.dma_start(out=wq_sb, in_=wq[:, :].rearrange("(c p) n -> p c n", p=P))

            # ---- small constants (tiny DMAs; before the remaining weights) ----
            rq_sb = smp.tile([P, DCH], f32, tag="rq", bufs=1)
            nc.sync.dma_start(out=rq_sb, in_=rq[:].rearrange("(c p) -> p c", p=P))
            bq_sb = smp.tile([P, DCH], f32, tag="bq", bufs=1)
            nc.sync.dma_start(out=bq_sb, in_=bq[:].rearrange("(c p) -> p c", p=P))
            rk_sb = smp.tile([P, DCH], f32, tag="rk", bufs=1)
            nc.sync.dma_start(out=rk_sb, in_=rk[:].rearrange("(c p) -> p c", p=P))
            bk_sb = smp.tile([P, DCH], f32, tag="bk", bufs=1)
            nc.sync.dma_start(out=bk_sb, in_=bk[:].rearrange("(c p) -> p c", p=P))
            rv_sb = smp.tile([P, 1], f32, tag="rv", bufs=1)
            nc.sync.dma_start(out=rv_sb, in_=rv[:].rearrange("(p c) -> p c", c=1))
            eps1_sb = smp.tile([P, 1], f32, tag="eps1", bufs=1)
            nc.sync.dma_start(out=eps1_sb, in_=eps1[:].rearrange("(p c) -> p c", c=1))
            b1_sb = smp.tile([P, DFF // P], f32, tag="b1", bufs=1)
            nc.sync.dma_start(out=b1_sb, in_=b1p[:].rearrange("(c p) -> p c", p=P))

            nc.sync.dma_start(
                out=xtb[:, :, SQ:S],
                in_=xt[:, SQ:S].rearrange("(c p) s -> p c s", p=P))
            wv_sb = sp.tile([P, KC, D], fp8, tag="slotE")
            nc.sync.dma_start(out=wv_sb, in_=wv[:, :].rearrange("(c p) n -> p c n", p=P))
            wk_sb = mp.tile([P, KC, D], fp8, tag="wres")
            nc.sync.dma_start(out=wk_sb, in_=wk[:, :].rearrange("(c p) n -> p c n", p=P))

            ident_bf = mp.tile([P, P], bf16, tag="ident_bf")
            make_identity(nc, ident_bf)
            ident_f32 = mp.tile([P, P], f32, tag="ident_f32")
            make_identity(nc, ident_f32)

            # prepay the exp ACT table load
            warm = mp.tile([P, 1], f32, tag="warm")
            nc.vector.memset(warm, 0.0)
            nc.scalar.activation(warm, warm, AF.Exp)
            ebias_t = mp.tile([P, 1], f32, tag="ebias")
            nc.vector.memset(ebias_t, E_BIAS)

            qtb = mp.tile([P, H, SQ], bf16, tag="slotA")
            nc.vector.memset(qtb, 0.0)
            vaug = mp.tile([P, SCH, H * (DK + 1)], fp8, tag="slotD")
            va_view = vaug.rearrange("p s (h w) -> p s h w", w=DK + 1)
            nc.vector.memset(va_view[:, :, :, DK:DK + 1], 1.0)

            # ================= QT (feature-major [D, SQ], bf16) =================
            with nc.named_scope("qkv"):
                for dch in range(DCH):
                    pts = [ps.tile([P, NW], f32, tag="mm", name=f"pt{i}") for i in range(2)]
                    for kp in range(KC // 2):
                        for n in range(2):
                            nc.tensor.matmul(
                                pts[n],
                                wq_sb[:, 2 * kp:2 * kp + 2, dch * P:(dch + 1) * P],
                                xtb[:, 2 * kp:2 * kp + 2, n * NW:(n + 1) * NW],
                                start=(kp == 0), stop=(kp == KC // 2 - 1),
                                perf_mode=DR)
                    for n in range(2):
                        nc.vector.tensor_scalar(
                            qtb[0:64, 2 * dch, n * NW:(n + 1) * NW],
                            pts[n][0:64, :],
                            rq_sb[0:64, dch:dch + 1], bq_sb[0:64, dch:dch + 1],
                            op0=Alu.mult, op1=Alu.add)
                        nc.vector.tensor_scalar(
                            qtb[64:128, 2 * dch + 1, n * NW:(n + 1) * NW],
                            pts[n][64:128, :],
                            rq_sb[64:128, dch:dch + 1], bq_sb[64:128, dch:dch + 1],
                            op0=Alu.mult, op1=Alu.add)

                # ---- V (token-major, fp8 = S_V*V, +ones col per head) ----
                for sch in range(SCH):
                    pts = [ps.tile([P, NW], f32, tag="mm", name=f"pt{i}") for i in range(2)]
                    for kp in range(KC // 2):
                        for n in range(2):
                            nc.tensor.matmul(
                                pts[n],
                                xtb[:, 2 * kp:2 * kp + 2, sch * P:(sch + 1) * P],
                                wv_sb[:, 2 * kp:2 * kp + 2, n * NW:(n + 1) * NW],
                                start=(kp == 0), stop=(kp == KC // 2 - 1),
                                perf_mode=DR)
                    for n in range(2):
                        h0 = n * (NW // DK)
                        nc.vector.tensor_scalar(
                            va_view[:, sch, h0:h0 + 8, 0:DK],
                            pts[n].rearrange("p (h w) -> p h w", w=DK),
                            rv_sb[:, 0:1], None, op0=Alu.mult)

            # ================= KT chunk + attention heads, pipelined ============
            concatT = mp.tile([P, DCH, SQ], fp8, tag="slotF")

            def kt_chunk(dch):
                ktch = ktp.tile([P, S], bf16, tag="ktch", name=f"ktch{dch}")
                for half in range(2):
                    for n in range(2):
                        pt = ps.tile([P, NW], f32, tag="mm", name="ptk")
                        for kp in range(KC // 2):
                            nc.tensor.matmul(
                                pt,
                                wk_sb[:, 2 * kp:2 * kp + 2, dch * P:(dch + 1) * P],
                                xtb[:, 2 * kp:2 * kp + 2,
                                    (half * 2 + n) * NW:(half * 2 + n + 1) * NW],
                                start=(kp == 0), stop=(kp == KC // 2 - 1),
                                perf_mode=DR)
                        nc.vector.tensor_scalar(
                            ktch[:, (half * 2 + n) * NW:(half * 2 + n + 1) * NW],
                            pt, rk_sb[:, dch:dch + 1], bk_sb[:, dch:dch + 1],
                            op0=Alu.mult, op1=Alu.add)
                return ktch

            def attn_heads(hp, ktch):
                # heads 2*hp, 2*hp+1 against full-seq K chunk ktch
                at2 = {}
                for j in range(2):
                    for q in range(4):
                        at2[(j, q)] = at2p.tile([P, P], bf16, tag="at2",
                                                name=f"at2_{j}_{q}")
                for hsub in range(2):
                    h = 2 * hp + hsub
                    ets = []
                    for j in range(2):
                        et = sp.tile([P, SCH, NW], fp8, tag="slotE", name=f"et{j}")
                        for sch in range(SCH):
                            pt = ps.tile([P, NW], f32, tag="mm", name="ptsc")
                            nc.tensor.matmul(
                                pt,
                                ktch[:, sch * P:(sch + 1) * P],
                                qtb[:, h, j * NW:(j + 1) * NW],
                                start=True, stop=True)
                            nc.scalar.activation(
                                et[:, sch, :], pt, AF.Exp,
                                bias=ebias_t[:, 0:1],
                                scale=float(1.0 / np.sqrt(DK)))
                        ets.append(et)
                    for j in range(2):
                        et = ets[j]
                        for q in range(4):
                            pat = psat.tile([P, DK + 1], f32, tag="at")
                            for sp_ in range(SCH // 2):
                                nc.tensor.matmul(
                                    pat,
                                    et[:, 2 * sp_:2 * sp_ + 2, q * P:(q + 1) * P],
                                    vaug[:, 2 * sp_:2 * sp_ + 2,
                                         h * (DK + 1):(h + 1) * (DK + 1)],
                                    start=(sp_ == 0), stop=(sp_ == SCH // 2 - 1),
                                    perf_mode=DR)
                            rec = smp.tile([P, 1], f32, tag="rec")
                            nc.vector.reciprocal(rec, pat[:, DK:DK + 1])
                            nc.vector.tensor_scalar_mul(
                                at2[(j, q)][:, hsub * DK:(hsub + 1) * DK],
                                pat[:, 0:DK], rec)
                for j in range(2):
                    for q in range(4):
                        ptr = pstr.tile([P, P], bf16, tag="tr")
                        nc.tensor.transpose(ptr, at2[(j, q)], ident_bf)
                        nc.vector.tensor_copy(
                            concatT[:, hp, j * NW + q * P: j * NW + (q + 1) * P],
                            ptr)

            with nc.named_scope("attn"):
                prev = kt_chunk(0)
                for dch in range(1, DCH):
                    cur = kt_chunk(dch)
                    attn_heads(dch - 1, prev)
                    prev = cur
                attn_heads(DCH - 1, prev)

            # ================= O-projection + LN1 =================
            norm1 = mp.tile([P, SQCH, D], f32, tag="slotA")
            norm1T = mp.tile([P, DCH, SQ], bf16, tag="slotC")
            wo_sb = mp.tile([P, KC, D], fp8, tag="wres")  # reuse wk slot
            nc.sync.dma_start(out=wo_sb, in_=wo[:, :].rearrange("(c p) n -> p c n", p=P))
            a1_b = mp.tile([P, D], f32, tag="a1_b")
            nc.sync.dma_start(out=a1_b, in_=bcast(a1p[:], D))
            g1_b = mp.tile([P, D], f32, tag="g1_b")
            nc.sync.dma_start(out=g1_b, in_=bcast(g1p[:], D))

            with nc.named_scope("o_ln1"):
                for sq in range(SQCH):
                    pts = [ps.tile([P, NW], f32, tag="mm", name=f"pt{i}") for i in range(2)]
                    for cp in range(DCH // 2):
                        for n in range(2):
                            nc.tensor.matmul(
                                pts[n],
                                concatT[:, 2 * cp:2 * cp + 2, sq * P:(sq + 1) * P],
                                wo_sb[:, 2 * cp:2 * cp + 2, n * NW:(n + 1) * NW],
                                start=(cp == 0), stop=(cp == DCH // 2 - 1),
                                perf_mode=DR)
                    xh_t = tokp.tile([P, D], f32, tag="tokf32")
                    nc.sync.dma_start(out=xh_t, in_=xh[sq * P:(sq + 1) * P, :])
                    s1 = norm1[:, sq, :]
                    for n in range(2):
                        nc.vector.tensor_add(
                            s1[:, n * NW:(n + 1) * NW], pts[n],
                            xh_t[:, n * NW:(n + 1) * NW])
                    # LN1 (scale-invariant: s1 carries c1; eps pre-scaled)
                    stats = smp.tile([P, 2, 6], f32, tag="stats")
                    nc.vector.bn_stats(stats[:, 0, :], s1[:, 0:NW])
                    nc.vector.bn_stats(stats[:, 1, :], s1[:, NW:2 * NW])
                    mv = smp.tile([P, 2], f32, tag="mv")
                    nc.vector.bn_aggr(mv, stats)
                    mean_t = smp.tile([P, 1], f32, tag="mean")
                    std_t = smp.tile([P, 1], f32, tag="std")
                    rec_t = smp.tile([P, 1], f32, tag="recs")
                    nc.scalar.activation(std_t, mv[:, 1:2], AF.Sqrt,
                                         scale=float(D / (D - 1)))
                    nc.vector.tensor_scalar_add(std_t, std_t, eps1_sb[:, 0:1])
                    nc.vector.reciprocal(rec_t, std_t)
                    nc.vector.tensor_copy(mean_t, mv[:, 0:1])
                    nc.vector.tensor_scalar(
                        s1, s1, mean_t, rec_t, op0=Alu.subtract, op1=Alu.mult)
                    # transpose plain z -> norm1T; affine applied after, in place
                    for dch in range(DCH):
                        ptr = pstr.tile([P, P], f32, tag="tr")
                        nc.tensor.transpose(
                            ptr, norm1[:, sq, dch * P:(dch + 1) * P], ident_f32)
                        nc.scalar.activation(
                            norm1T[:, dch, sq * P:(sq + 1) * P], ptr, AF.Copy)
                    nc.gpsimd.tensor_mul(norm1[:, sq, :], norm1[:, sq, :], a1_b)
                    nc.gpsimd.tensor_add(norm1[:, sq, :], norm1[:, sq, :], g1_b)

            # ================= FFN =================
            relu0 = mp.tile([P, 16, SQ], bf16, tag="slotB")   # dff chunks 0..15
            relu1 = mp.tile([P, 16, SQ], bf16, tag="slotD")   # dff chunks 16..31
            with nc.named_scope("ffn1"):
                for w8 in range(8):
                    w1_sb = sp.tile([P, KC, NW], bf16, tag="slotE")
                    nc.sync.dma_start(
                        out=w1_sb,
                        in_=w1[:, w8 * NW:(w8 + 1) * NW].rearrange(
                            "(c p) n -> p c n", p=P))
                    for dsub in range(4):
                        dff_ch = w8 * 4 + dsub
                        tgt = relu0 if dff_ch < 16 else relu1
                        tch = dff_ch % 16
                        pts = [ps.tile([P, NW], f32, tag="mm", name=f"pt{i}") for i in range(2)]
                        for kc in range(KC):
                            for n in range(2):
                                nc.tensor.matmul(
                                    pts[n],
                                    w1_sb[:, kc, dsub * P:(dsub + 1) * P],
                                    norm1T[:, kc, n * NW:(n + 1) * NW],
                                    start=(kc == 0), stop=(kc == KC - 1))
                        for n in range(2):
                            nc.scalar.activation(
                                tgt[:, tch, n * NW:(n + 1) * NW], pts[n],
                                AF.Relu, bias=b1_sb[:, dff_ch:dff_ch + 1])

            s2a = mp.tile([P, SQCH, 512], f32, tag="slotC")   # features 0:512
            s2b = mp.tile([P, SQCH, 512], f32, tag="slotF")   # features 512:1024
            a2_b = mp.tile([P, D], f32, tag="a1_b")
            nc.sync.dma_start(out=a2_b, in_=bcast(a2p[:], D))
            g2_b = mp.tile([P, D], f32, tag="g1_b")
            nc.sync.dma_start(out=g2_b, in_=bcast(g2p[:], D))
            with nc.named_scope("ffn2"):
                st2 = [smp.tile([P, 2, 6], f32, tag="stats2", name=f"st2_{i}",
                                bufs=8) for i in range(SQCH)]
                for ncol in range(4):
                    w2_sb = sp.tile([P, DFF // P, 256], bf16, tag="slotE")
                    nc.sync.dma_start(
                        out=w2_sb,
                        in_=w2[:, ncol * 256:(ncol + 1) * 256].rearrange(
                            "(c p) n -> p c n", p=P))
                    tgt = s2a if ncol < 2 else s2b
                    tcol = (ncol % 2) * 256
                    fcol = ncol * 256
                    for sq in range(SQCH):
                        pt = ps.tile([P, 256], f32, tag="mm")
                        for kc in range(DFF // P):
                            lhs = relu0 if kc < 16 else relu1
                            nc.tensor.matmul(
                                pt,
                                lhs[:, kc % 16, sq * P:(sq + 1) * P],
                                w2_sb[:, kc, :],
                                start=(kc == 0), stop=(kc == DFF // P - 1))
                        nc.vector.tensor_add(
                            tgt[:, sq, tcol:tcol + 256], pt,
                            norm1[:, sq, fcol:fcol + 256])
                        if ncol == 1:
                            nc.vector.bn_stats(st2[sq][:, 0, :], s2a[:, sq, :])
                        if ncol == 3:
                            nc.vector.bn_stats(st2[sq][:, 1, :], s2b[:, sq, :])
                            mv = smp.tile([P, 2], f32, tag="mv")
                            nc.vector.bn_aggr(mv, st2[sq])
                            mean_t = smp.tile([P, 1], f32, tag="mean")
                            std_t = smp.tile([P, 1], f32, tag="std")
                            rec_t = smp.tile([P, 1], f32, tag="recs")
                            nc.scalar.activation(std_t, mv[:, 1:2], AF.Sqrt,
                                                 scale=float(D / (D - 1)))
                            nc.vector.tensor_scalar_add(std_t, std_t, float(EPS))
                            nc.vector.reciprocal(rec_t, std_t)
                            nc.vector.tensor_copy(mean_t, mv[:, 0:1])
                            for tgt2, alo in ((s2a, 0), (s2b, 512)):
                                nc.vector.tensor_scalar(
                                    tgt2[:, sq, :], tgt2[:, sq, :], mean_t, rec_t,
                                    op0=Alu.subtract, op1=Alu.mult)
                                nc.vector.tensor_mul(tgt2[:, sq, :], tgt2[:, sq, :],
                                                     a2_b[:, alo:alo + 512])
                                nc.vector.tensor_add(tgt2[:, sq, :], tgt2[:, sq, :],
                                                     g2_b[:, alo:alo + 512])
                            nc.sync.dma_start(
                                out=out[sq * P:(sq + 1) * P, 0:512],
                                in_=s2a[:, sq, :])
                            nc.sync.dma_start(
                                out=out[sq * P:(sq + 1) * P, 512:1024],
                                in_=s2b[:, sq, :])

    nc.compile()
    return nc


def _get_program():
    global _PROG
    if _PROG is None:
        _PROG = _build_program()
    return _PROG


def _q8(a, s):
    return np.clip(np.asarray(a, np.float32) * s, -240.0, 240.0).astype(F8)


def make_in_maps(x, Wq, bq, Wk, bk, Wv, bv, Wo, bo, alpha1, bias1, alpha2,
                 bias2, W1, b1, W2, b2):
    f32 = np.float32
    x = np.asarray(x, f32)
    Wq = np.asarray(Wq, f32); Wk = np.asarray(Wk, f32)
    Wv = np.asarray(Wv, f32); Wo = np.asarray(Wo, f32)
    W1 = np.asarray(W1, f32); W2 = np.asarray(W2, f32)
    bv = np.asarray(bv, f32); bo = np.asarray(bo, f32)

    sx = 240.0 / max(float(np.abs(x).max()), 1e-30)
    swq = 240.0 / np.maximum(np.abs(Wq).max(axis=0), 1e-30)   # per col
    swk = 240.0 / np.maximum(np.abs(Wk).max(axis=0), 1e-30)
    swv = 240.0 / max(float(np.abs(Wv).max()), 1e-30)          # per tensor
    swo = 240.0 / max(float(np.abs(Wo).max()), 1e-30)
    c1 = S_V * swo

    shared = {
        "wq": _q8(Wq, swq[None, :]), "wk": _q8(Wk, swk[None, :]),
        "wv": _q8(Wv, swv), "wo": _q8(Wo, swo),
        "w1": (np.asarray(alpha1, f32)[:, None] * W1).astype(BF16),
        "w2": W2.astype(BF16),
        "rq": (1.0 / (sx * swq)).astype(f32),
        "rk": (1.0 / (sx * swk)).astype(f32),
        "rv": np.full((P,), S_V / (sx * swv), f32),
        "eps1": np.full((P,), c1 * EPS, f32),
        "bq": np.asarray(bq, f32), "bk": np.asarray(bk, f32),
        "b1": (np.asarray(b1, f32) + np.asarray(bias1, f32) @ W1),
        "alpha1": np.asarray(alpha1, f32),
        "beta1": (np.asarray(bias1, f32) + np.asarray(b2, f32)),
        "alpha2": np.asarray(alpha2, f32),
        "beta2": np.asarray(bias2, f32),
    }
    resid_bias = (bo + bv @ Wo)[None, :]
    in_maps = []
    for c in range(NCORES):
        b, j = c // 2, c % 2
        xb = x[b]
        if j == 0:
            xt_np = xb.T
        else:
            xt_np = np.concatenate([xb[SQ:].T, xb[:SQ].T], axis=1)
        m = dict(shared)
        m["xt"] = _q8(xt_np, sx)
        m["xh"] = np.ascontiguousarray(
            (xb[j * SQ:(j + 1) * SQ] + resid_bias) * c1, dtype=f32)
        in_maps.append(m)
    return in_maps


def kernel(**inputs):
    from concourse.bass_utils import run_bass_kernel_spmd

    nc = _get_program()
    in_maps = make_in_maps(**inputs)
    res = run_bass_kernel_spmd(nc, in_maps, core_ids=list(range(NCORES)))
    out = np.empty((B, S, D), np.float32)
    for c in range(NCORES):
        b, j = c // 2, c % 2
        out[b, j * SQ:(j + 1) * SQ, :] = res.results[c]["out"]
    return out
